# revision 5
# baseline (speedup 1.0000x reference)
"""Conv2d(128->256, 3x3, pad=1) over (32,128,56,56), data-parallel across 8
NeuronCores (4 images per core).

Per core: conv = 9 shifted accumulating matmuls per output tile.
  - contraction K = Cin = 128 (partition dim)
  - stationary lhsT = W^T[ci, co_tile] per (ky,kx)  -> [128, 128]
  - moving rhs = zero-padded input pixels [128, 8 rows, 56 cols] (N = 448)
  - PSUM accumulates the 9 (ky,kx) tap contributions
Input is staged in SBUF as [128, 58, 58] with zeroed 1-pixel border, so every
tap is a uniform full-tile matmul (also satisfies fp32r ISA restrictions:
even innermost counts, unit steps).
Bias is added during the PSUM->SBUF copy (ScalarE/VectorE alternating).
"""

import numpy as np
import ml_dtypes

import concourse.mybir as mybir
import concourse.tile as tile
from concourse import bacc
from concourse.bass_utils import run_bass_kernel_spmd

N_CORES = 8
B, CIN, H, W = 32, 128, 56, 56
COUT, R, S = 256, 3, 3
BL = B // N_CORES          # images per core
NCOT = COUT // 128         # Cout tiles of 128
YCHUNK = 8                 # output rows per matmul tile
NYC = H // YCHUNK
HP, WP = H + 2, W + 2      # padded input tile dims

MM_DT = mybir.dt.float32r  # matmul operand dtype on device
MM_NP = np.float32         # matching numpy dtype for host-side cast

_cache = {}


def _build():
    if "nc" in _cache:
        return _cache["nc"]
    nc = bacc.Bacc("TRN2", target_bir_lowering=False, debug=False)
    f32 = mybir.dt.float32
    x_d = nc.dram_tensor("x", [BL, CIN, H, W], MM_DT, kind="ExternalInput").ap()
    w_d = nc.dram_tensor("w", [CIN, NCOT, R, S, 128], MM_DT, kind="ExternalInput").ap()
    b_d = nc.dram_tensor("b", [128, NCOT], f32, kind="ExternalInput").ap()
    y_d = nc.dram_tensor("y", [BL, COUT, H, W], f32, kind="ExternalOutput").ap()

    with tile.TileContext(nc) as tc:
        with (
            tc.tile_pool(name="consts", bufs=1) as cpool,
            tc.tile_pool(name="xin", bufs=2) as xpool,
            tc.tile_pool(name="yout", bufs=2) as opool,
            tc.tile_pool(name="ps", bufs=8, space="PSUM") as pspool,
        ):
            w_sb = cpool.tile([CIN, NCOT, R, S, 128], MM_DT)
            nc.sync.dma_start(w_sb[:], w_d[:])
            b_sb = cpool.tile([128, NCOT], f32)
            nc.sync.dma_start(b_sb[:], b_d[:])

            for img in range(BL):
                x_sb = xpool.tile([CIN, HP, WP], MM_DT, name=f"x_sb_{img}", tag="x_sb")
                # zero the 1-pixel border (pad ring)
                xv = x_sb.bitcast(f32) if MM_DT == mybir.dt.float32r else x_sb
                nc.gpsimd.memset(xv[:, 0, :], 0.0)
                nc.gpsimd.memset(xv[:, HP - 1, :], 0.0)
                nc.vector.memset(xv[:, 1 : HP - 1, 0], 0.0)
                nc.vector.memset(xv[:, 1 : HP - 1, WP - 1], 0.0)
                nc.sync.dma_start(x_sb[:, 1 : HP - 1, 1 : WP - 1], x_d[img])
                for cot in range(NCOT):
                    o_sb = opool.tile(
                        [128, H, W], f32, name=f"o_sb_{img}_{cot}", tag="o_sb"
                    )
                    for yc in range(NYC):
                        y0 = YCHUNK * yc
                        ps = pspool.tile(
                            [128, YCHUNK, W], f32, name=f"ps_{img}_{cot}_{yc}", tag="ps"
                        )
                        for ky in range(R):
                            for kx in range(S):
                                nc.tensor.matmul(
                                    ps[:],
                                    w_sb[:, cot, ky, kx, :],
                                    x_sb[:, y0 + ky : y0 + ky + YCHUNK, kx : kx + W],
                                    start=(ky == 0 and kx == 0),
                                    stop=(ky == R - 1 and kx == S - 1),
                                )
                        # PSUM -> SBUF with fused bias add; alternate engines
                        if yc % 2 == 0:
                            nc.scalar.activation(
                                o_sb[:, y0 : y0 + YCHUNK, :],
                                ps[:],
                                mybir.ActivationFunctionType.Identity,
                                bias=b_sb[:, cot : cot + 1],
                            )
                        else:
                            nc.vector.tensor_scalar_add(
                                o_sb[:, y0 : y0 + YCHUNK, :],
                                ps[:],
                                b_sb[:, cot : cot + 1],
                            )
                    nc.sync.dma_start(y_d[img, 128 * cot : 128 * (cot + 1)], o_sb[:])

    nc.compile()
    _cache["nc"] = nc
    return nc


def kernel(inputs, weight, bias):
    nc = _build()
    x = np.asarray(inputs).astype(MM_NP)
    # weight (co, ci, ky, kx) -> (ci, cot, ky, kx, co_in_tile)
    w = np.ascontiguousarray(
        np.asarray(weight)
        .reshape(NCOT, 128, CIN, R, S)
        .transpose(2, 0, 3, 4, 1)
        .astype(MM_NP)
    )
    b = np.ascontiguousarray(
        np.asarray(bias).astype(np.float32).reshape(NCOT, 128).T
    )
    in_maps = [
        {"x": np.ascontiguousarray(x[c * BL : (c + 1) * BL]), "w": w, "b": b}
        for c in range(N_CORES)
    ]
    res = run_bass_kernel_spmd(nc, in_maps, core_ids=list(range(N_CORES)))
    return np.concatenate([res.results[c]["y"] for c in range(N_CORES)], axis=0)


# revision 6
# speedup vs baseline: 1.1615x; 1.1615x over previous
"""Conv2d(128->256, 3x3, pad=1) over (32,128,56,56), data-parallel across 8
NeuronCores (4 images per core).

Per core: conv = 9 shifted accumulating matmuls per output tile.
  - contraction K = Cin = 128 (partition dim)
  - stationary lhsT = W^T[ci, co_tile] per (ky,kx)  -> [128, 128] bf16
  - moving rhs = input pixels [128, <=8 rows, <=56 cols] (N <= 448)
  - PSUM accumulates the 9 (ky,kx) taps; padding handled by clipping each
    tap's matmul to the valid rectangle (center tap goes first with
    start=True and covers the full tile, so partial-range taps accumulate
    on top via PSUM's per-element has_written bits).
Bias is added during the PSUM->SBUF copy (ScalarE/VectorE alternating).
DMA queue split: input loads on GpSimd (SWDGE), weights + half the output
stores on Sync, other half of stores on Scalar (2nd HWDGE ring) so loads
and stores never queue behind each other.
"""

import numpy as np
import ml_dtypes

import concourse.mybir as mybir
import concourse.tile as tile
from concourse import bacc
from concourse.bass_utils import run_bass_kernel_spmd

N_CORES = 8
B, CIN, H, W = 32, 128, 56, 56
COUT, R, S = 256, 3, 3
BL = B // N_CORES          # images per core
NCOT = COUT // 128         # Cout tiles of 128
YCHUNK = 8                 # output rows per matmul tile
NYC = H // YCHUNK

MM_DT = mybir.dt.bfloat16
MM_NP = ml_dtypes.bfloat16

_cache = {}


def _build():
    if "nc" in _cache:
        return _cache["nc"]
    nc = bacc.Bacc("TRN2", target_bir_lowering=False, debug=False)
    f32 = mybir.dt.float32
    x_d = nc.dram_tensor("x", [BL, CIN, H, W], MM_DT, kind="ExternalInput").ap()
    w_d = nc.dram_tensor("w", [CIN, NCOT, R, S, 128], MM_DT, kind="ExternalInput").ap()
    b_d = nc.dram_tensor("b", [128, NCOT], f32, kind="ExternalInput").ap()
    y_d = nc.dram_tensor("y", [BL, COUT, H, W], f32, kind="ExternalOutput").ap()

    with tile.TileContext(nc) as tc:
        with (
            tc.tile_pool(name="consts", bufs=1) as cpool,
            tc.tile_pool(name="xin", bufs=2) as xpool,
            tc.tile_pool(name="yout", bufs=2) as opool,
            tc.tile_pool(name="ps", bufs=8, space="PSUM") as pspool,
        ):
            w_sb = cpool.tile([CIN, NCOT, R, S, 128], MM_DT)
            nc.sync.dma_start(w_sb[:], w_d[:])
            b_sb = cpool.tile([128, NCOT], f32)
            nc.sync.dma_start(b_sb[:], b_d[:])

            for img in range(BL):
                x_sb = xpool.tile([CIN, H, W], MM_DT, name=f"x_sb_{img}", tag="x_sb")
                # split the image load so the first chunks' rows land earlier
                nc.gpsimd.dma_start(x_sb[:, 0:32, :], x_d[img, :, 0:32, :])
                nc.gpsimd.dma_start(x_sb[:, 32:H, :], x_d[img, :, 32:H, :])
                for cot in range(NCOT):
                    o_sb = opool.tile(
                        [128, H, W], f32, name=f"o_sb_{img}_{cot}", tag="o_sb"
                    )
                    for yc in range(NYC):
                        y0 = YCHUNK * yc
                        ps = pspool.tile(
                            [128, YCHUNK, W], f32, name=f"ps_{img}_{cot}_{yc}", tag="ps"
                        )
                        # center tap first: full-tile write with start=True
                        nc.tensor.matmul(
                            ps[:],
                            w_sb[:, cot, 1, 1, :],
                            x_sb[:, y0 : y0 + YCHUNK, :],
                            start=True,
                            stop=False,
                        )
                        for ky in range(R):
                            for kx in range(S):
                                if ky == 1 and kx == 1:
                                    continue
                                oy0 = max(0, 1 - ky - y0)
                                oy1 = min(YCHUNK, H + 1 - y0 - ky)
                                ox0 = max(0, 1 - kx)
                                ox1 = min(W, W + 1 - kx)
                                nc.tensor.matmul(
                                    ps[:, oy0:oy1, ox0:ox1],
                                    w_sb[:, cot, ky, kx, :],
                                    x_sb[
                                        :,
                                        y0 + oy0 + ky - 1 : y0 + oy1 + ky - 1,
                                        ox0 + kx - 1 : ox1 + kx - 1,
                                    ],
                                    start=False,
                                    stop=(ky == R - 1 and kx == S - 1),
                                )
                        # PSUM -> SBUF with fused bias add; alternate engines
                        if yc % 2 == 0:
                            nc.scalar.activation(
                                o_sb[:, y0 : y0 + YCHUNK, :],
                                ps[:],
                                mybir.ActivationFunctionType.Identity,
                                bias=b_sb[:, cot : cot + 1],
                            )
                        else:
                            nc.vector.tensor_scalar_add(
                                o_sb[:, y0 : y0 + YCHUNK, :],
                                ps[:],
                                b_sb[:, cot : cot + 1],
                            )
                        # store finished halves (rows 0:28 after chunk 3,
                        # rows 28:56 after chunk 6), alternating HWDGE rings
                        if yc == 3:
                            eng = nc.sync if (img + cot) % 2 == 0 else nc.scalar
                            eng.dma_start(
                                y_d[img, 128 * cot : 128 * (cot + 1), 0:28, :],
                                o_sb[:, 0:28, :],
                            )
                        elif yc == NYC - 1:
                            eng = nc.scalar if (img + cot) % 2 == 0 else nc.sync
                            eng.dma_start(
                                y_d[img, 128 * cot : 128 * (cot + 1), 28:H, :],
                                o_sb[:, 28:H, :],
                            )

    nc.compile()
    _cache["nc"] = nc
    return nc


def kernel(inputs, weight, bias):
    nc = _build()
    x = np.asarray(inputs).astype(MM_NP)
    # weight (co, ci, ky, kx) -> (ci, cot, ky, kx, co_in_tile)
    w = np.ascontiguousarray(
        np.asarray(weight)
        .reshape(NCOT, 128, CIN, R, S)
        .transpose(2, 0, 3, 4, 1)
        .astype(MM_NP)
    )
    b = np.ascontiguousarray(
        np.asarray(bias).astype(np.float32).reshape(NCOT, 128).T
    )
    in_maps = [
        {"x": np.ascontiguousarray(x[c * BL : (c + 1) * BL]), "w": w, "b": b}
        for c in range(N_CORES)
    ]
    res = run_bass_kernel_spmd(nc, in_maps, core_ids=list(range(N_CORES)))
    return np.concatenate([res.results[c]["y"] for c in range(N_CORES)], axis=0)


# revision 7
# speedup vs baseline: 1.2303x; 1.0593x over previous
"""Conv2d(128->256, 3x3, pad=1) over (32,128,56,56), data-parallel across 8
NeuronCores (4 images per core).

Per core: conv = 9 shifted accumulating matmuls per output tile.
  - contraction K = Cin = 128 (partition dim)
  - stationary lhsT = W^T[ci, co_tile] per (ky,kx)  -> [128, 128] bf16
  - moving rhs = input pixels [128, <=8 rows, <=56 cols] (N <= 448)
  - PSUM accumulates the 9 (ky,kx) taps; padding handled by clipping each
    tap's matmul to the valid rectangle (center tap goes first with
    start=True and covers the full tile, so partial-range taps accumulate
    on top via PSUM's per-element has_written bits).
Bias is added during the PSUM->SBUF copy (ScalarE/VectorE alternating).

Latency structure:
  - first image is loaded in row-quarters on the Sync HWDGE ring while the
    weights load in cot-halves on the Scalar HWDGE ring, so the first
    matmul can start as soon as quarter 0 + weight half 0 land;
  - a handful of zero dummy matmuls bridge the PE from the preamble to the
    first data-dependent matmul so the HAM clock-gate warms early;
  - images 1..3 prefetch on the GpSimd SWDGE queue;
  - output stores go out in row-quarters alternating Sync/Scalar rings so
    the final store before the exit barrier is small.
"""

import numpy as np
import ml_dtypes

import concourse.mybir as mybir
import concourse.tile as tile
from concourse import bacc
from concourse.bass_utils import run_bass_kernel_spmd

N_CORES = 8
B, CIN, H, W = 32, 128, 56, 56
COUT, R, S = 256, 3, 3
BL = B // N_CORES          # images per core
NCOT = COUT // 128         # Cout tiles of 128
YCHUNK = 8                 # output rows per matmul tile
NYC = H // YCHUNK

MM_DT = mybir.dt.bfloat16
MM_NP = ml_dtypes.bfloat16

NWARM = 6                  # dummy matmuls to bridge PE from preamble to data
X0_SPLITS = [0, 16, 32, 44, 56]       # first-image load quarters (rows)
OUT_SPLITS = {1: (0, 14), 3: (14, 28), 5: (28, 42), 6: (42, 56)}  # yc -> store rows

_cache = {}


def _build():
    if "nc" in _cache:
        return _cache["nc"]
    nc = bacc.Bacc("TRN2", target_bir_lowering=False, debug=False)
    f32 = mybir.dt.float32
    x_d = nc.dram_tensor("x", [BL, CIN, H, W], MM_DT, kind="ExternalInput").ap()
    w_d = nc.dram_tensor("w", [CIN, NCOT, R, S, 128], MM_DT, kind="ExternalInput").ap()
    b_d = nc.dram_tensor("b", [128, NCOT], f32, kind="ExternalInput").ap()
    y_d = nc.dram_tensor("y", [BL, COUT, H, W], f32, kind="ExternalOutput").ap()

    with tile.TileContext(nc) as tc:
        with (
            tc.tile_pool(name="consts", bufs=1) as cpool,
            tc.tile_pool(name="xin", bufs=2) as xpool,
            tc.tile_pool(name="yout", bufs=2) as opool,
            tc.tile_pool(name="ps", bufs=8, space="PSUM") as pspool,
        ):
            # --- PE prewarm: zero matmuls with no DMA dependency ---
            warm_x = cpool.tile([128, 512], MM_DT)
            nc.vector.memset(warm_x[:], 0.0)
            warm_ps = pspool.tile([128, 512], f32, tag="ps")
            for _ in range(NWARM):
                nc.tensor.matmul(
                    warm_ps[:], warm_x[:, 0:128], warm_x[:], start=True, stop=True
                )

            # --- constants + first image, on parallel HWDGE rings ---
            w_sb = cpool.tile([CIN, NCOT, R, S, 128], MM_DT)
            for cot in range(NCOT):
                nc.scalar.dma_start(w_sb[:, cot], w_d[:, cot])
            b_sb = cpool.tile([128, NCOT], f32)
            nc.gpsimd.dma_start(b_sb[:], b_d[:])

            x_tiles = []
            x0 = xpool.tile([CIN, H, W], MM_DT, name="x_sb_0", tag="x_sb")
            for r0, r1 in zip(X0_SPLITS, X0_SPLITS[1:]):
                nc.sync.dma_start(x0[:, r0:r1, :], x_d[0, :, r0:r1, :])
            x_tiles.append(x0)

            for img in range(BL):
                if img > 0:
                    x_sb = xpool.tile(
                        [CIN, H, W], MM_DT, name=f"x_sb_{img}", tag="x_sb"
                    )
                    nc.gpsimd.dma_start(x_sb[:], x_d[img])
                else:
                    x_sb = x_tiles[0]
                for cot in range(NCOT):
                    o_sb = opool.tile(
                        [128, H, W], f32, name=f"o_sb_{img}_{cot}", tag="o_sb"
                    )
                    for yc in range(NYC):
                        y0 = YCHUNK * yc
                        ps = pspool.tile(
                            [128, YCHUNK, W], f32, name=f"ps_{img}_{cot}_{yc}", tag="ps"
                        )
                        # center tap first: full-tile write with start=True
                        nc.tensor.matmul(
                            ps[:],
                            w_sb[:, cot, 1, 1, :],
                            x_sb[:, y0 : y0 + YCHUNK, :],
                            start=True,
                            stop=False,
                        )
                        for ky in range(R):
                            for kx in range(S):
                                if ky == 1 and kx == 1:
                                    continue
                                oy0 = max(0, 1 - ky - y0)
                                oy1 = min(YCHUNK, H + 1 - y0 - ky)
                                ox0 = max(0, 1 - kx)
                                ox1 = min(W, W + 1 - kx)
                                nc.tensor.matmul(
                                    ps[:, oy0:oy1, ox0:ox1],
                                    w_sb[:, cot, ky, kx, :],
                                    x_sb[
                                        :,
                                        y0 + oy0 + ky - 1 : y0 + oy1 + ky - 1,
                                        ox0 + kx - 1 : ox1 + kx - 1,
                                    ],
                                    start=False,
                                    stop=(ky == R - 1 and kx == S - 1),
                                )
                        # PSUM -> SBUF with fused bias add; alternate engines
                        if yc % 2 == 0:
                            nc.scalar.activation(
                                o_sb[:, y0 : y0 + YCHUNK, :],
                                ps[:],
                                mybir.ActivationFunctionType.Identity,
                                bias=b_sb[:, cot : cot + 1],
                            )
                        else:
                            nc.vector.tensor_scalar_add(
                                o_sb[:, y0 : y0 + YCHUNK, :],
                                ps[:],
                                b_sb[:, cot : cot + 1],
                            )
                        # store finished row-quarters, alternating HWDGE rings
                        if yc in OUT_SPLITS:
                            r0, r1 = OUT_SPLITS[yc]
                            q = list(OUT_SPLITS).index(yc)
                            eng = nc.sync if (img + cot + q) % 2 == 0 else nc.scalar
                            eng.dma_start(
                                y_d[img, 128 * cot : 128 * (cot + 1), r0:r1, :],
                                o_sb[:, r0:r1, :],
                            )

    nc.compile()
    _cache["nc"] = nc
    return nc


def kernel(inputs, weight, bias):
    nc = _build()
    x = np.asarray(inputs).astype(MM_NP)
    # weight (co, ci, ky, kx) -> (ci, cot, ky, kx, co_in_tile)
    w = np.ascontiguousarray(
        np.asarray(weight)
        .reshape(NCOT, 128, CIN, R, S)
        .transpose(2, 0, 3, 4, 1)
        .astype(MM_NP)
    )
    b = np.ascontiguousarray(
        np.asarray(bias).astype(np.float32).reshape(NCOT, 128).T
    )
    in_maps = [
        {"x": np.ascontiguousarray(x[c * BL : (c + 1) * BL]), "w": w, "b": b}
        for c in range(N_CORES)
    ]
    res = run_bass_kernel_spmd(nc, in_maps, core_ids=list(range(N_CORES)))
    return np.concatenate([res.results[c]["y"] for c in range(N_CORES)], axis=0)
